# revision 12
# baseline (speedup 1.0000x reference)
"""Trainium2 Bass kernel for nn_MultiHeadAttention_68590627717608.

Strategy: tensor-parallel over the 16 attention heads -> 2 heads per core on 8
NeuronCores. Each core computes E / Ep / attv / A / avAp / Hnext for its two
heads plus a partial output projection; the host gathers head-sharded outputs
and sums the output-projection partials.

The host pre-transposes q/k/v to [D, S] so every on-device matmul has its
contraction dimension on the SBUF partition axis. The attention matmuls
(E, E^T, attv@v, out-proj) run with bf16 operands (fp32 PSUM accumulation);
the q/k projections and the metric-tensor/ResLayer path stay fp32 so the A
and avAp outputs are fp32-exact. Instead of transposing attv on the PE, the
kernel recomputes E^T directly (swapping the lhsT/rhs roles of the same
tiles) and folds the softmax normalization into a post-scale of Hnext^T.
"""

import numpy as np

# Problem dimensions (fixed by the reference).
B, S, D, H = 1, 2048, 1024, 16
DEPTH = D // H          # 64
A_DFF = 1024
NRES = 2
SLOPE = 0.2
POW_EPS = 1e-9
LN_EPS = 1e-6
N_CORES = 8
HPC = H // N_CORES      # heads per core = 2
P = HPC * DEPTH         # 128 = head-block width per core

F32 = np.float32

_CACHE = {}


def build_program(s_len, has_bias_row):
    from contextlib import ExitStack

    import concourse.bacc as bacc
    from concourse import mybir
    from concourse.tile import TileContext

    f32 = mybir.dt.float32
    bf16 = mybir.dt.bfloat16
    Alu = mybir.AluOpType
    Act = mybir.ActivationFunctionType
    Ax = mybir.AxisListType

    assert s_len % 1024 == 0
    SH = s_len // 2          # s-half for the projection phase
    NDC = D // 128           # 8 contraction chunks for projections
    NSC = s_len // 128       # 128-row s-chunks
    NTB = s_len // 512       # 512-wide blocks

    nc = bacc.Bacc("TRN2", target_bir_lowering=False, debug=False,
                   num_devices=N_CORES)

    din = {}

    def dram_in(name, shape, dt=f32):
        din[name] = nc.dram_tensor(name, list(shape), dt, kind="ExternalInput")
        return din[name]

    qT_d = dram_in("qT", (D, s_len))
    kT_d = dram_in("kT", (D, s_len))
    vT_d = dram_in("vT", (D, s_len), bf16)
    wq_d = dram_in("wq_s", (D, P))
    wk_d = dram_in("wk_s", (D, P))
    wv_d = dram_in("wv_s", (D, P), bf16)
    wqb_d = dram_in("wq_bs", (1, P))
    wkb_d = dram_in("wk_bs", (1, P))
    wvb_d = dram_in("wv_bs", (1, P), bf16)
    wo_d = dram_in("wo_s", (P, D), bf16)
    w1_d = dram_in("rl_w1", (NRES, DEPTH, A_DFF))
    b1_d = dram_in("rl_b1c", (NRES, A_DFF // 128, 128, 1))
    w2_d = dram_in("rl_w2", (NRES, A_DFF, DEPTH))
    b2_d = dram_in("rl_b2r", (NRES, 1, DEPTH))
    g_d = dram_in("rl_g_bc", (NRES, DEPTH, DEPTH))
    beta_d = dram_in("rl_beta_bc", (NRES, DEPTH, DEPTH))
    avec_d = dram_in("avec_bc", (DEPTH, DEPTH))
    pw_d = dram_in("pw_col", (DEPTH, 1))
    ident_d = dram_in("ident", (128, 128))
    ident_bf_d = dram_in("ident_bf", (128, 128), bf16)
    if has_bias_row:
        biasrow_d = dram_in("bias_row", (1, s_len))
        biascol_d = dram_in("bias_colT", (128, NSC))

    E_d = nc.dram_tensor("E_out", [HPC, s_len, s_len], f32, kind="ExternalOutput")
    Ep_d = nc.dram_tensor("Ep_out", [HPC, s_len, s_len], f32, kind="ExternalOutput")
    attv_d = nc.dram_tensor("attv_out", [HPC, s_len, s_len], f32, kind="ExternalOutput")
    A_d = nc.dram_tensor("A_out", [HPC, DEPTH, DEPTH], f32, kind="ExternalOutput")
    av_d = nc.dram_tensor("av_out", [HPC, DEPTH, 1], f32, kind="ExternalOutput")
    op_d = nc.dram_tensor("out_part", [s_len, D], f32, kind="ExternalOutput")

    mm = nc.tensor.matmul
    dma = nc.sync.dma_start

    with TileContext(nc) as tc, ExitStack() as stack:
        pers = stack.enter_context(tc.tile_pool(name="pers", bufs=1))
        consts = stack.enter_context(tc.tile_pool(name="consts", bufs=1))

        ident = consts.tile([128, 128], f32, tag="ident", name="ident")
        dma(ident[:], ident_d[:, :])
        ident_bf = consts.tile([128, 128], bf16, tag="ident_bf", name="ident_bf")
        dma(ident_bf[:], ident_bf_d[:, :])
        ones = consts.tile([1, 512], f32, tag="ones", name="ones")
        nc.vector.memset(ones[:], 1.0)
        ones_bf = consts.tile([1, 512], bf16, tag="ones_bf", name="ones_bf")
        nc.vector.memset(ones_bf[:], 1.0)
        zcol = consts.tile([128, 1], f32, tag="zcol", name="zcol")
        nc.vector.memset(zcol[:], 0.0)
        lneps_c = consts.tile([DEPTH, 1], f32, tag="lneps", name="lneps")
        nc.vector.memset(lneps_c[:], LN_EPS)
        poweps_c = consts.tile([DEPTH, 1], f32, tag="poweps", name="poweps")
        nc.vector.memset(poweps_c[:], POW_EPS)

        wo_sb = consts.tile([P, D], bf16, tag="wo", name="wo")
        dma(wo_sb[:], wo_d[:, :])

        # persistent per-head activations
        khT = [pers.tile([DEPTH, s_len], bf16, tag=f"khT{h}", name=f"khT{h}")
               for h in range(HPC)]
        qhsc = [pers.tile([DEPTH, s_len], bf16, tag=f"qhsc{h}", name=f"qhsc{h}")
                for h in range(HPC)]
        qhT_full = pers.tile([128, s_len], f32, tag="qhT_full", name="qhT_full")
        vhT_full = pers.tile([128, s_len], bf16, tag="vhT_full", name="vhT_full")
        vh_sd = pers.tile([128, NSC * P], bf16, tag="vh_sd", name="vh_sd")
        qh_sd = pers.tile([128, NSC * P], f32, tag="qh_sd", name="qh_sd")
        HnT = pers.tile([P, s_len], bf16, tag="HnT", name="HnT")
        inv_all = pers.tile([128, HPC * NSC], f32, tag="inv_all", name="inv_all")

        if has_bias_row:
            biasrow_sb = consts.tile([1, s_len], f32, tag="biasrow", name="biasrow")
            dma(biasrow_sb[:], biasrow_d[:, :])
            biascol_sb = consts.tile([128, NSC], f32, tag="biascol", name="biascol")
            dma(biascol_sb[:], biascol_d[:, :])
            bias_bc = pers.tile([128, s_len], f32, tag="bias_bc", name="bias_bc")

        # ---------------- projection phase ----------------
        with tc.tile_pool(name="pw_pool", bufs=1) as wpool:
            wq_sb = wpool.tile([128, NDC * P], f32, tag="wq", name="wq")
            wk_sb = wpool.tile([128, NDC * P], f32, tag="wk", name="wk")
            wv_sb = wpool.tile([128, NDC * P], bf16, tag="wv", name="wv")
            for c in range(NDC):
                sl = slice(c * 128, (c + 1) * 128)
                dma(wq_sb[:, c * P:(c + 1) * P], wq_d[sl, :])
                dma(wk_sb[:, c * P:(c + 1) * P], wk_d[sl, :])
                dma(wv_sb[:, c * P:(c + 1) * P], wv_d[sl, :])
            wqb = wpool.tile([1, P], f32, tag="wqb", name="wqb")
            wkb = wpool.tile([1, P], f32, tag="wkb", name="wkb")
            wvb = wpool.tile([1, P], bf16, tag="wvb", name="wvb")
            dma(wqb[:], wqb_d[:, :])
            dma(wkb[:], wkb_d[:, :])
            dma(wvb[:], wvb_d[:, :])

            if has_bias_row:
                with tc.tile_pool(name="bb_ps", bufs=2, space="PSUM") as bbps:
                    for tb in range(NTB):
                        bps = bbps.tile([128, 512], f32, tag="bb", name="bb")
                        mm(bps[:], ones[:, 0:128],
                           biasrow_sb[:, tb * 512:(tb + 1) * 512],
                           start=True, stop=True)
                        nc.vector.tensor_copy(bias_bc[:, tb * 512:(tb + 1) * 512],
                                              bps[:])

            with (
                tc.tile_pool(name="qT_pool", bufs=2) as qpool,
                tc.tile_pool(name="kT_pool", bufs=2) as kpool,
                tc.tile_pool(name="vT_pool", bufs=2) as vpool,
                tc.tile_pool(name="prj_ps", bufs=1, space="PSUM") as pps,
                tc.tile_pool(name="str_ps", bufs=1, space="PSUM") as strps,
            ):
                nblk = SH // 512
                for half in range(2):
                    ssl = slice(half * SH, (half + 1) * SH)
                    qh_ps = pps.tile([128, SH], f32, tag="qh_ps", name="qh_ps")
                    kh_ps = pps.tile([128, SH], f32, tag="kh_ps", name="kh_ps")
                    vh_ps = pps.tile([128, SH], f32, tag="vh_ps", name="vh_ps")
                    for c in range(NDC):
                        qt = qpool.tile([128, SH], f32, tag="qT", name="qT")
                        dma(qt[:], qT_d[c * 128:(c + 1) * 128, ssl])
                        kt = kpool.tile([128, SH], f32, tag="kT", name="kT")
                        dma(kt[:], kT_d[c * 128:(c + 1) * 128, ssl])
                        vt = vpool.tile([128, SH], bf16, tag="vT", name="vT")
                        dma(vt[:], vT_d[c * 128:(c + 1) * 128, ssl])
                        wslq = wq_sb[:, c * P:(c + 1) * P]
                        wslk = wk_sb[:, c * P:(c + 1) * P]
                        wslv = wv_sb[:, c * P:(c + 1) * P]
                        for nb in range(nblk):
                            nsl = slice(nb * 512, (nb + 1) * 512)
                            mm(qh_ps[:, nsl], wslq, qt[:, nsl],
                               start=(c == 0), stop=False)
                            mm(kh_ps[:, nsl], wslk, kt[:, nsl],
                               start=(c == 0), stop=False)
                            mm(vh_ps[:, nsl], wslv, vt[:, nsl],
                               start=(c == 0), stop=False)
                    for nb in range(nblk):
                        nsl = slice(nb * 512, (nb + 1) * 512)
                        mm(qh_ps[:, nsl], wqb[:], ones[:], start=False, stop=True)
                        mm(kh_ps[:, nsl], wkb[:], ones[:], start=False, stop=True)
                        mm(vh_ps[:, nsl], wvb[:], ones_bf[:], start=False, stop=True)
                    nc.vector.tensor_copy(qhT_full[:, ssl], qh_ps[:])
                    nc.vector.tensor_copy(vhT_full[:, ssl], vh_ps[:])
                    for h in range(HPC):
                        psl = slice(h * DEPTH, (h + 1) * DEPTH)
                        nc.vector.tensor_copy(khT[h][:, ssl], kh_ps[psl, :])

                    # transpose projections into [t, d] layouts
                    nj = SH // 128
                    for jj in range(nj):
                        j = half * nj + jj
                        isl = slice(half * SH + jj * 128,
                                    half * SH + (jj + 1) * 128)
                        tq = strps.tile([128, 128], f32, tag="tq", name="tq")
                        nc.tensor.transpose(tq[:], qhT_full[:, isl], ident[:])
                        nc.vector.tensor_copy(qh_sd[:, j * P:(j + 1) * P], tq[:])
                        tv = strps.tile([128, 128], bf16, tag="tv", name="tv")
                        nc.tensor.transpose(tv[:], vhT_full[:, isl], ident_bf[:])
                        nc.scalar.copy(vh_sd[:, j * P:(j + 1) * P], tv[:])

        # ---------------- A path: metric tensor, ResLayers, plga ----------------
        with (
            tc.tile_pool(name="rl_w", bufs=1) as rlw,
            tc.tile_pool(name="a_sb", bufs=1) as asb,
            tc.tile_pool(name="a_ps", bufs=1, space="PSUM") as aps,
            tc.tile_pool(name="a_sc", bufs=2) as asc,
        ):
            w1_sb = [rlw.tile([DEPTH, A_DFF], f32, tag=f"w1_{i}", name=f"w1_{i}")
                     for i in range(NRES)]
            w2_sb = [rlw.tile([128, (A_DFF // 128) * DEPTH], f32, tag=f"w2_{i}",
                              name=f"w2_{i}") for i in range(NRES)]
            b1_sb = [rlw.tile([128, A_DFF // 128], f32, tag=f"b1_{i}",
                              name=f"b1_{i}") for i in range(NRES)]
            b2_sb = [rlw.tile([1, DEPTH], f32, tag=f"b2_{i}", name=f"b2_{i}")
                     for i in range(NRES)]
            g_sb = [rlw.tile([DEPTH, DEPTH], f32, tag=f"g_{i}", name=f"g_{i}")
                    for i in range(NRES)]
            be_sb = [rlw.tile([DEPTH, DEPTH], f32, tag=f"be_{i}", name=f"be_{i}")
                     for i in range(NRES)]
            for i in range(NRES):
                dma(w1_sb[i][:], w1_d[i])
                for c in range(A_DFF // 128):
                    dma(w2_sb[i][:, c * DEPTH:(c + 1) * DEPTH],
                        w2_d[i, c * 128:(c + 1) * 128, :])
                    dma(b1_sb[i][:, c:c + 1], b1_d[i, c])
                dma(b2_sb[i][:], b2_d[i])
                dma(g_sb[i][:], g_d[i])
                dma(be_sb[i][:], beta_d[i])
            avec_sb = rlw.tile([DEPTH, DEPTH], f32, tag="avec", name="avec")
            dma(avec_sb[:], avec_d[:, :])
            pw_sb = rlw.tile([DEPTH, 1], f32, tag="pw", name="pw")
            dma(pw_sb[:], pw_d[:, :])

            a_ps = aps.tile([128, 128], f32, tag="a_full", name="a_full")
            for j in range(NSC):
                jsl = slice(j * P, (j + 1) * P)
                mm(a_ps[:], qh_sd[:, jsl], qh_sd[:, jsl],
                   start=(j == 0), stop=(j == NSC - 1))

            for h in range(HPC):
                hsl = slice(h * DEPTH, (h + 1) * DEPTH)
                A_sb = asb.tile([DEPTH, DEPTH], f32, tag=f"A_{h}", name=f"A_{h}")
                nc.vector.tensor_copy(A_sb[:], a_ps[hsl, hsl])

                for i in range(NRES):
                    t_ps = aps.tile([DEPTH, DEPTH], f32, tag="t_ps", name="t_ps")
                    nc.tensor.transpose(t_ps[:], A_sb[:], ident[0:DEPTH, 0:DEPTH])
                    AinT = asc.tile([DEPTH, DEPTH], f32, tag="AinT", name="AinT")
                    nc.vector.tensor_copy(AinT[:], t_ps[:])

                    hdnT = asc.tile([128, (A_DFF // 128) * DEPTH], f32,
                                    tag="hdnT", name="hdnT")
                    for fc in range(A_DFF // 128):
                        hd_ps = aps.tile([128, DEPTH], f32, tag="hd_ps",
                                         name="hd_ps", bufs=2)
                        mm(hd_ps[:], w1_sb[i][:, fc * 128:(fc + 1) * 128], AinT[:],
                           start=True, stop=True)
                        nc.scalar.activation(hdnT[:, fc * DEPTH:(fc + 1) * DEPTH],
                                             hd_ps[:], Act.Relu,
                                             bias=b1_sb[i][:, fc:fc + 1])

                    h2_ps = aps.tile([DEPTH, DEPTH], f32, tag="h2_ps", name="h2_ps")
                    for fc in range(A_DFF // 128):
                        fsl = slice(fc * DEPTH, (fc + 1) * DEPTH)
                        mm(h2_ps[:], hdnT[:, fsl], w2_sb[i][:, fsl],
                           start=(fc == 0), stop=False)
                    mm(h2_ps[:], ones[:, 0:DEPTH], b2_sb[i][:],
                       start=False, stop=False)
                    mm(h2_ps[:], ident[0:DEPTH, 0:DEPTH], A_sb[:],
                       start=False, stop=True)

                    ssum = asc.tile([DEPTH, 1], f32, tag="ssum", name="ssum")
                    nc.vector.tensor_reduce(ssum[:], h2_ps[:], Ax.X, Alu.add)
                    mean = asc.tile([DEPTH, 1], f32, tag="mean", name="mean")
                    nc.vector.tensor_scalar_mul(mean[:], ssum[:], 1.0 / DEPTH)
                    xc = asc.tile([DEPTH, DEPTH], f32, tag="xc", name="xc")
                    nc.vector.tensor_scalar(xc[:], h2_ps[:], mean[:], None,
                                            Alu.subtract)
                    xsq = asc.tile([DEPTH, DEPTH], f32, tag="xsq", name="xsq")
                    ssq = asc.tile([DEPTH, 1], f32, tag="ssq", name="ssq")
                    nc.scalar.activation(xsq[:], xc[:], Act.Square,
                                         bias=zcol[0:DEPTH, :], accum_out=ssq[:])
                    lnv = asc.tile([DEPTH, 1], f32, tag="lnv", name="lnv")
                    nc.scalar.activation(lnv[:], ssq[:], Act.Ln,
                                         scale=1.0 / DEPTH, bias=lneps_c[:])
                    rstd = asc.tile([DEPTH, 1], f32, tag="rstd", name="rstd")
                    nc.scalar.activation(rstd[:], lnv[:], Act.Exp, scale=-0.5,
                                         bias=zcol[0:DEPTH, :])
                    A_sb = asb.tile([DEPTH, DEPTH], f32, tag=f"A_{h}_l{i}",
                                    name=f"A_{h}_l{i}")
                    nc.vector.scalar_tensor_tensor(A_sb[:], xc[:], rstd[:],
                                                   g_sb[i][:], Alu.mult, Alu.mult)
                    nc.vector.tensor_tensor(A_sb[:], A_sb[:], be_sb[i][:], Alu.add)

                dma(A_d[h], A_sb[:])

                absA = asc.tile([DEPTH, DEPTH], f32, tag="absA", name="absA")
                nc.vector.scalar_tensor_tensor(absA[:], A_sb[:], -1.0, A_sb[:],
                                               Alu.mult, Alu.max)
                L = asc.tile([DEPTH, DEPTH], f32, tag="L", name="L")
                nc.scalar.activation(L[:], absA[:], Act.Ln, bias=poweps_c[:])
                lt_ps = aps.tile([DEPTH, DEPTH], f32, tag="lt_ps", name="lt_ps")
                nc.tensor.transpose(lt_ps[:], L[:], ident[0:DEPTH, 0:DEPTH])
                PT = asc.tile([DEPTH, DEPTH], f32, tag="PT", name="PT")
                nc.vector.tensor_scalar(PT[:], lt_ps[:], pw_sb[:], None, Alu.mult)
                ApT = asc.tile([DEPTH, DEPTH], f32, tag="ApT", name="ApT")
                nc.scalar.activation(ApT[:], PT[:], Act.Exp, bias=zcol[0:DEPTH, :])
                wApT = asc.tile([DEPTH, DEPTH], f32, tag="wApT", name="wApT")
                nc.vector.tensor_tensor(wApT[:], ApT[:], avec_sb[:], Alu.mult)
                av_sb = asc.tile([DEPTH, 1], f32, tag=f"av_{h}", name=f"av_{h}")
                nc.vector.tensor_reduce(av_sb[:], wApT[:], Ax.X, Alu.add)
                dma(av_d[h], av_sb[:])

                # scaled q head (bf16) for the E / E^T matmuls
                nc.vector.tensor_scalar(qhsc[h][:], qhT_full[hsl, :], av_sb[:],
                                        None, Alu.mult)

        # ---------------- main attention loops ----------------
        with (
            tc.tile_pool(name="e_ps", bufs=2, space="PSUM") as eps_pool,
            tc.tile_pool(name="et_ps", bufs=2, space="PSUM") as etps_pool,
            tc.tile_pool(name="hn_ps", bufs=2, space="PSUM") as hnps_pool,
            tc.tile_pool(name="it_ps", bufs=1, space="PSUM") as itps_pool,
            tc.tile_pool(name="bc_ps", bufs=1, space="PSUM") as bcps_pool,
            tc.tile_pool(name="row_sb", bufs=2) as row_pool,
            tc.tile_pool(name="sm_sb", bufs=2) as sm_pool,
            tc.tile_pool(name="tp_sb", bufs=3) as tp_pool,
        ):
            for h in range(HPC):
                for sb in range(NTB):
                    # ---- s-orientation pass: 4 row-chunks of 128 s each
                    for c4 in range(4):
                        sc = sb * 4 + c4
                        ssl = slice(sc * 128, (sc + 1) * 128)
                        icol = inv_all[:, h * NSC + sc: h * NSC + sc + 1]
                        e_sb = row_pool.tile([128, s_len], f32, tag="e_sb",
                                             name="e_sb")
                        for tb in range(NTB):
                            nsl = slice(tb * 512, (tb + 1) * 512)
                            e_ps = eps_pool.tile([128, 512], f32, tag="e_ps",
                                                 name="e_ps")
                            mm(e_ps[:], qhsc[h][:, ssl], khT[h][:, nsl],
                               start=True, stop=True)
                            if tb % 2 == 0:
                                nc.vector.tensor_copy(e_sb[:, nsl], e_ps[:])
                            else:
                                nc.scalar.copy(e_sb[:, nsl], e_ps[:])
                        dma(E_d[h, ssl, :], e_sb[:])

                        ep_sb = row_pool.tile([128, s_len], f32, tag="ep_sb",
                                              name="ep_sb")
                        nc.vector.scalar_tensor_tensor(ep_sb[:], e_sb[:], SLOPE,
                                                       e_sb[:], Alu.mult, Alu.max)
                        if has_bias_row:
                            nc.vector.tensor_tensor(ep_sb[:], ep_sb[:],
                                                    bias_bc[:], Alu.add)
                        dma(Ep_d[h, ssl, :], ep_sb[:])

                        p_sb = sm_pool.tile([128, s_len], f32, tag="p_sb",
                                            name="p_sb")
                        d_col = sm_pool.tile([128, 1], f32, tag="d_col",
                                             name="d_col")
                        nc.scalar.activation(p_sb[:], ep_sb[:], Act.Exp,
                                             bias=zcol[:], accum_out=d_col[:])
                        nc.vector.reciprocal(icol, d_col[:])
                        attv_sb = sm_pool.tile([128, s_len], f32, tag="attv_sb",
                                               name="attv_sb")
                        nc.vector.tensor_scalar(attv_sb[:], p_sb[:], icol,
                                                None, Alu.mult)
                        dma(attv_d[h, ssl, :], attv_sb[:])

                    # ---- transposed pass for this 512-wide s-block
                    bsl = slice(sb * 512, (sb + 1) * 512)
                    inv_row = tp_pool.tile([1, 512], f32, tag="inv_row",
                                           name="inv_row")
                    for c4 in range(4):
                        sc = sb * 4 + c4
                        it_ps = itps_pool.tile([1, 128], f32, tag="it_ps",
                                               name="it_ps")
                        nc.tensor.transpose(
                            it_ps[:], inv_all[:, h * NSC + sc: h * NSC + sc + 1],
                            ident[:])
                        nc.vector.tensor_copy(inv_row[:, c4 * 128:(c4 + 1) * 128],
                                              it_ps[:])
                    bc_ps = bcps_pool.tile([DEPTH, 512], f32, tag="bc_ps",
                                           name="bc_ps")
                    mm(bc_ps[:], ones[:, 0:DEPTH], inv_row[:], start=True,
                       stop=True)
                    inv_bc = tp_pool.tile([DEPTH, 512], f32, tag="inv_bc",
                                          name="inv_bc")
                    nc.vector.tensor_copy(inv_bc[:], bc_ps[:])

                    hn_ps = hnps_pool.tile([DEPTH, 512], f32, tag="hn_ps",
                                           name="hn_ps")
                    for tb in range(NSC):
                        tsl = slice(tb * 128, (tb + 1) * 128)
                        et_ps = etps_pool.tile([128, 512], f32, tag="et_ps",
                                               name="et_ps")
                        mm(et_ps[:], khT[h][:, tsl], qhsc[h][:, bsl],
                           start=True, stop=True)
                        et_sb = tp_pool.tile([128, 512], f32, tag="et_sb",
                                             name="et_sb")
                        nc.scalar.copy(et_sb[:], et_ps[:])
                        lre = tp_pool.tile([128, 512], f32, tag="lre", name="lre")
                        nc.vector.scalar_tensor_tensor(lre[:], et_sb[:], SLOPE,
                                                       et_sb[:], Alu.mult, Alu.max)
                        if has_bias_row:
                            nc.vector.tensor_scalar(
                                lre[:], lre[:], biascol_sb[:, tb:tb + 1], None,
                                Alu.add)
                        pT = tp_pool.tile([128, 512], bf16, tag="pT", name="pT")
                        nc.scalar.activation(pT[:], lre[:], Act.Exp, bias=zcol[:])
                        mm(hn_ps[:], vh_sd[:, tb * P + h * DEPTH:
                                           tb * P + (h + 1) * DEPTH], pT[:],
                           start=(tb == 0), stop=(tb == NSC - 1))
                    nc.vector.scalar_tensor_tensor(
                        HnT[h * DEPTH:(h + 1) * DEPTH, bsl], hn_ps[:], 1.0,
                        inv_bc[:], Alu.mult, Alu.mult)

            # ---------------- output projection (partial) ----------------
            for sc in range(NSC):
                ssl = slice(sc * 128, (sc + 1) * 128)
                o_sb = row_pool.tile([128, D], f32, tag="o_sb", name="o_sb")
                for nb in range(D // 512):
                    nsl = slice(nb * 512, (nb + 1) * 512)
                    o_ps = eps_pool.tile([128, 512], f32, tag="e_ps", name="o_ps")
                    mm(o_ps[:], HnT[:, ssl], wo_sb[:, nsl], start=True, stop=True)
                    nc.vector.tensor_copy(o_sb[:, nsl], o_ps[:])
                dma(op_d[ssl, :], o_sb[:])

    nc.compile()
    return nc


def _prepare_inputs(q, k, v, mask, wq_w, wq_b, wk_w, wk_b, wv_w, wv_b, wo_w, wo_b,
                    rl_w1, rl_b1, rl_w2, rl_b2, rl_g, rl_beta, pw, a_vec, ba,
                    s_len):
    """Build the 8 per-core input maps (host-side sharding / layout prep)."""
    import ml_dtypes
    bf = ml_dtypes.bfloat16
    f = lambda x: np.ascontiguousarray(np.asarray(x, dtype=F32))
    fb = lambda x: np.ascontiguousarray(np.asarray(x, dtype=F32).astype(bf))
    qT = f(np.asarray(q, dtype=F32).reshape(s_len, D).T)
    kT = f(np.asarray(k, dtype=F32).reshape(s_len, D).T)
    vT = fb(np.asarray(v, dtype=F32).reshape(s_len, D).T)
    ident = np.eye(128, dtype=F32)
    ident_bf = np.eye(128).astype(bf)
    b1c = f(np.asarray(rl_b1).reshape(NRES, A_DFF // 128, 128, 1))
    b2r = f(np.asarray(rl_b2).reshape(NRES, 1, DEPTH))
    g_bc = f(np.broadcast_to(np.asarray(rl_g)[:, None, :], (NRES, DEPTH, DEPTH)))
    beta_bc = f(np.broadcast_to(np.asarray(rl_beta)[:, None, :],
                                (NRES, DEPTH, DEPTH)))
    avec_bc = f(np.broadcast_to(np.asarray(a_vec)[None, :], (DEPTH, DEPTH)))
    pw_col = f(np.asarray(pw).reshape(DEPTH, 1))
    bias_row = (np.asarray(ba, dtype=F32)
                + np.asarray(mask, dtype=F32).reshape(1, s_len) * F32(-1e9))
    has_bias_row = bool(np.any(bias_row != 0))

    in_maps = []
    for c in range(N_CORES):
        cols = slice(c * P, (c + 1) * P)
        m = {
            "qT": qT, "kT": kT, "vT": vT,
            "wq_s": f(np.asarray(wq_w)[:, cols]),
            "wk_s": f(np.asarray(wk_w)[:, cols]),
            "wv_s": fb(np.asarray(wv_w, dtype=F32)[:, cols]),
            "wq_bs": f(np.asarray(wq_b)[cols].reshape(1, P)),
            "wk_bs": f(np.asarray(wk_b)[cols].reshape(1, P)),
            "wv_bs": fb(np.asarray(wv_b, dtype=F32)[cols].reshape(1, P)),
            "wo_s": fb(np.asarray(wo_w, dtype=F32)[cols, :]),
            "rl_w1": f(rl_w1), "rl_b1c": b1c, "rl_w2": f(rl_w2), "rl_b2r": b2r,
            "rl_g_bc": g_bc, "rl_beta_bc": beta_bc,
            "avec_bc": avec_bc, "pw_col": pw_col,
            "ident": ident, "ident_bf": ident_bf,
        }
        if has_bias_row:
            m["bias_row"] = f(bias_row)
            m["bias_colT"] = f(bias_row.reshape(-1, 128).T)
        in_maps.append(m)
    return in_maps, has_bias_row


def _ensure_trn_backend():
    """The device run needs jax's axon TRN backend. If the surrounding process
    pinned JAX_PLATFORMS=cpu (e.g. to run the jax reference), lift that before
    jax/concourse initialize, and reset jax backends if they were already
    initialized without the TRN devices."""
    import os
    import sys
    jp = os.environ.get("JAX_PLATFORMS")
    if jp is not None and "axon" not in jp and jp != "":
        os.environ.pop("JAX_PLATFORMS", None)
    if "jax" in sys.modules:
        try:
            import jax
            if not any("NC_" in str(d) for d in jax.devices()):
                jax.clear_backends()
        except Exception:
            try:
                import jax
                jax.clear_backends()
            except Exception:
                pass


def kernel(q, k, v, mask, wq_w, wq_b, wk_w, wk_b, wv_w, wv_b, wo_w, wo_b,
           rl_w1, rl_b1, rl_w2, rl_b2, rl_g, rl_beta, pw, a_vec, ba,
           _trace=False):
    _ensure_trn_backend()
    from concourse.bass_utils import run_bass_kernel_spmd

    s_len = int(np.asarray(q).shape[1])
    in_maps, has_bias_row = _prepare_inputs(
        q, k, v, mask, wq_w, wq_b, wk_w, wk_b, wv_w, wv_b, wo_w, wo_b,
        rl_w1, rl_b1, rl_w2, rl_b2, rl_g, rl_beta, pw, a_vec, ba, s_len)

    key = (s_len, has_bias_row)
    if key not in _CACHE:
        _CACHE[key] = build_program(s_len, has_bias_row)
    nc = _CACHE[key]

    res = run_bass_kernel_spmd(nc, in_maps, core_ids=list(range(N_CORES)),
                               trace=_trace)

    E = np.empty((1, H, s_len, s_len), dtype=F32)
    Ep = np.empty((1, H, s_len, s_len), dtype=F32)
    attv = np.empty((1, H, s_len, s_len), dtype=F32)
    A = np.empty((1, H, DEPTH, DEPTH), dtype=F32)
    avAp = np.empty((1, H, DEPTH), dtype=F32)
    out_acc = np.zeros((s_len, D), dtype=F32)
    for c in range(N_CORES):
        r = res.results[c]
        E[0, c * HPC:(c + 1) * HPC] = r["E_out"]
        Ep[0, c * HPC:(c + 1) * HPC] = r["Ep_out"]
        attv[0, c * HPC:(c + 1) * HPC] = r["attv_out"]
        A[0, c * HPC:(c + 1) * HPC] = r["A_out"]
        avAp[0, c * HPC:(c + 1) * HPC] = r["av_out"].reshape(HPC, DEPTH)
        out_acc += r["out_part"]
    out = (out_acc + np.asarray(wo_b, dtype=F32)).reshape(1, s_len, D)

    pw_out = np.asarray(pw, dtype=F32)
    ba_out = np.asarray(ba, dtype=F32)
    ret = (out, E, A, pw_out, attv, ba_out, avAp, Ep)
    if _trace:
        return ret, res
    return ret


# revision 13
# speedup vs baseline: 1.0333x; 1.0333x over previous
"""Trainium2 Bass kernel for nn_MultiHeadAttention_68590627717608.

Strategy: tensor-parallel over the 16 attention heads -> 2 heads per core on 8
NeuronCores. Each core computes E / Ep / attv / A / avAp / Hnext for its two
heads plus a partial output projection; the host gathers head-sharded outputs
and sums the output-projection partials.

The host pre-transposes q/k/v to [D, S] so every on-device matmul has its
contraction dimension on the SBUF partition axis. The attention matmuls
(E, E^T, attv@v, out-proj) run with bf16 operands (fp32 PSUM accumulation);
the q/k projections and the metric-tensor/ResLayer path stay fp32 so the A
and avAp outputs are fp32-exact. Instead of transposing attv on the PE, the
kernel recomputes E^T directly (swapping the lhsT/rhs roles of the same
tiles) and folds the softmax normalization into a post-scale of Hnext^T.
"""

import numpy as np

# Problem dimensions (fixed by the reference).
B, S, D, H = 1, 2048, 1024, 16
DEPTH = D // H          # 64
A_DFF = 1024
NRES = 2
SLOPE = 0.2
POW_EPS = 1e-9
LN_EPS = 1e-6
N_CORES = 8
HPC = H // N_CORES      # heads per core = 2
P = HPC * DEPTH         # 128 = head-block width per core

F32 = np.float32

_CACHE = {}


def build_program(s_len, has_bias_row):
    from contextlib import ExitStack

    import concourse.bacc as bacc
    from concourse import mybir
    from concourse.tile import TileContext

    f32 = mybir.dt.float32
    bf16 = mybir.dt.bfloat16
    Alu = mybir.AluOpType
    Act = mybir.ActivationFunctionType
    Ax = mybir.AxisListType

    assert s_len % 1024 == 0
    SH = s_len // 2          # s-half for the projection phase
    NDC = D // 128           # 8 contraction chunks for projections
    NSC = s_len // 128       # 128-row s-chunks
    NTB = s_len // 512       # 512-wide blocks

    nc = bacc.Bacc("TRN2", target_bir_lowering=False, debug=False,
                   num_devices=N_CORES)

    din = {}

    def dram_in(name, shape, dt=f32):
        din[name] = nc.dram_tensor(name, list(shape), dt, kind="ExternalInput")
        return din[name]

    qT_d = dram_in("qT", (D, s_len))
    kT_d = dram_in("kT", (D, s_len))
    vT_d = dram_in("vT", (D, s_len), bf16)
    wq_d = dram_in("wq_s", (D, P))
    wk_d = dram_in("wk_s", (D, P))
    wv_d = dram_in("wv_s", (D, P), bf16)
    wqb_d = dram_in("wq_bs", (1, P))
    wkb_d = dram_in("wk_bs", (1, P))
    wvb_d = dram_in("wv_bs", (1, P), bf16)
    wo_d = dram_in("wo_s", (P, D), bf16)
    w1_d = dram_in("rl_w1", (NRES, DEPTH, A_DFF))
    b1_d = dram_in("rl_b1c", (NRES, A_DFF // 128, 128, 1))
    w2_d = dram_in("rl_w2", (NRES, A_DFF, DEPTH))
    b2_d = dram_in("rl_b2r", (NRES, 1, DEPTH))
    g_d = dram_in("rl_g_bc", (NRES, DEPTH, DEPTH))
    beta_d = dram_in("rl_beta_bc", (NRES, DEPTH, DEPTH))
    avec_d = dram_in("avec_bc", (DEPTH, DEPTH))
    pw_d = dram_in("pw_col", (DEPTH, 1))
    ident_d = dram_in("ident", (128, 128))
    ident_bf_d = dram_in("ident_bf", (128, 128), bf16)
    if has_bias_row:
        biasrow_d = dram_in("bias_row", (1, s_len))
        biascol_d = dram_in("bias_colT", (128, NSC))

    E_d = nc.dram_tensor("E_out", [HPC, s_len, s_len], f32, kind="ExternalOutput")
    Ep_d = nc.dram_tensor("Ep_out", [HPC, s_len, s_len], f32, kind="ExternalOutput")
    attv_d = nc.dram_tensor("attv_out", [HPC, s_len, s_len], f32, kind="ExternalOutput")
    A_d = nc.dram_tensor("A_out", [HPC, DEPTH, DEPTH], f32, kind="ExternalOutput")
    av_d = nc.dram_tensor("av_out", [HPC, DEPTH, 1], f32, kind="ExternalOutput")
    op_d = nc.dram_tensor("out_part", [s_len, D], f32, kind="ExternalOutput")

    mm = nc.tensor.matmul
    dma = nc.sync.dma_start

    with TileContext(nc) as tc, ExitStack() as stack:
        pers = stack.enter_context(tc.tile_pool(name="pers", bufs=1))
        consts = stack.enter_context(tc.tile_pool(name="consts", bufs=1))

        ident = consts.tile([128, 128], f32, tag="ident", name="ident")
        dma(ident[:], ident_d[:, :])
        ident_bf = consts.tile([128, 128], bf16, tag="ident_bf", name="ident_bf")
        dma(ident_bf[:], ident_bf_d[:, :])
        ones = consts.tile([1, 512], f32, tag="ones", name="ones")
        nc.vector.memset(ones[:], 1.0)
        ones_bf = consts.tile([1, 512], bf16, tag="ones_bf", name="ones_bf")
        nc.vector.memset(ones_bf[:], 1.0)
        zcol = consts.tile([128, 1], f32, tag="zcol", name="zcol")
        nc.vector.memset(zcol[:], 0.0)
        lneps_c = consts.tile([DEPTH, 1], f32, tag="lneps", name="lneps")
        nc.vector.memset(lneps_c[:], LN_EPS)
        poweps_c = consts.tile([DEPTH, 1], f32, tag="poweps", name="poweps")
        nc.vector.memset(poweps_c[:], POW_EPS)

        wo_sb = consts.tile([P, D], bf16, tag="wo", name="wo")
        dma(wo_sb[:], wo_d[:, :])

        # persistent per-head activations
        khT = [pers.tile([DEPTH, s_len], bf16, tag=f"khT{h}", name=f"khT{h}")
               for h in range(HPC)]
        qhsc = [pers.tile([DEPTH, s_len], bf16, tag=f"qhsc{h}", name=f"qhsc{h}")
                for h in range(HPC)]
        qhT_full = pers.tile([128, s_len], f32, tag="qhT_full", name="qhT_full")
        vhT_full = pers.tile([128, s_len], bf16, tag="vhT_full", name="vhT_full")
        vh_sd = pers.tile([128, NSC * P], bf16, tag="vh_sd", name="vh_sd")
        qh_sd = pers.tile([128, NSC * P], f32, tag="qh_sd", name="qh_sd")
        HnT = pers.tile([P, s_len], bf16, tag="HnT", name="HnT")
        inv_all = pers.tile([128, HPC * NSC], f32, tag="inv_all", name="inv_all")

        if has_bias_row:
            biasrow_sb = consts.tile([1, s_len], f32, tag="biasrow", name="biasrow")
            dma(biasrow_sb[:], biasrow_d[:, :])
            biascol_sb = consts.tile([128, NSC], f32, tag="biascol", name="biascol")
            dma(biascol_sb[:], biascol_d[:, :])
            bias_bc = pers.tile([128, s_len], f32, tag="bias_bc", name="bias_bc")

        # ---------------- projection phase ----------------
        with tc.tile_pool(name="pw_pool", bufs=1) as wpool:
            wq_sb = wpool.tile([128, NDC * P], f32, tag="wq", name="wq")
            wk_sb = wpool.tile([128, NDC * P], f32, tag="wk", name="wk")
            wv_sb = wpool.tile([128, NDC * P], bf16, tag="wv", name="wv")
            for c in range(NDC):
                sl = slice(c * 128, (c + 1) * 128)
                dma(wq_sb[:, c * P:(c + 1) * P], wq_d[sl, :])
                dma(wk_sb[:, c * P:(c + 1) * P], wk_d[sl, :])
                dma(wv_sb[:, c * P:(c + 1) * P], wv_d[sl, :])
            wqb = wpool.tile([1, P], f32, tag="wqb", name="wqb")
            wkb = wpool.tile([1, P], f32, tag="wkb", name="wkb")
            wvb = wpool.tile([1, P], bf16, tag="wvb", name="wvb")
            dma(wqb[:], wqb_d[:, :])
            dma(wkb[:], wkb_d[:, :])
            dma(wvb[:], wvb_d[:, :])

            if has_bias_row:
                with tc.tile_pool(name="bb_ps", bufs=2, space="PSUM") as bbps:
                    for tb in range(NTB):
                        bps = bbps.tile([128, 512], f32, tag="bb", name="bb")
                        mm(bps[:], ones[:, 0:128],
                           biasrow_sb[:, tb * 512:(tb + 1) * 512],
                           start=True, stop=True)
                        nc.vector.tensor_copy(bias_bc[:, tb * 512:(tb + 1) * 512],
                                              bps[:])

            with (
                tc.tile_pool(name="qT_pool", bufs=2) as qpool,
                tc.tile_pool(name="kT_pool", bufs=2) as kpool,
                tc.tile_pool(name="vT_pool", bufs=2) as vpool,
                tc.tile_pool(name="prj_ps", bufs=1, space="PSUM") as pps,
                tc.tile_pool(name="str_ps", bufs=1, space="PSUM") as strps,
            ):
                nblk = SH // 512
                for half in range(2):
                    ssl = slice(half * SH, (half + 1) * SH)
                    qh_ps = pps.tile([128, SH], f32, tag="qh_ps", name="qh_ps")
                    kh_ps = pps.tile([128, SH], f32, tag="kh_ps", name="kh_ps")
                    vh_ps = pps.tile([128, SH], f32, tag="vh_ps", name="vh_ps")
                    for c in range(NDC):
                        qt = qpool.tile([128, SH], f32, tag="qT", name="qT")
                        dma(qt[:], qT_d[c * 128:(c + 1) * 128, ssl])
                        kt = kpool.tile([128, SH], f32, tag="kT", name="kT")
                        dma(kt[:], kT_d[c * 128:(c + 1) * 128, ssl])
                        vt = vpool.tile([128, SH], bf16, tag="vT", name="vT")
                        dma(vt[:], vT_d[c * 128:(c + 1) * 128, ssl])
                        wslq = wq_sb[:, c * P:(c + 1) * P]
                        wslk = wk_sb[:, c * P:(c + 1) * P]
                        wslv = wv_sb[:, c * P:(c + 1) * P]
                        for nb in range(nblk):
                            nsl = slice(nb * 512, (nb + 1) * 512)
                            mm(qh_ps[:, nsl], wslq, qt[:, nsl],
                               start=(c == 0), stop=False)
                            mm(kh_ps[:, nsl], wslk, kt[:, nsl],
                               start=(c == 0), stop=False)
                            mm(vh_ps[:, nsl], wslv, vt[:, nsl],
                               start=(c == 0), stop=False)
                    for nb in range(nblk):
                        nsl = slice(nb * 512, (nb + 1) * 512)
                        mm(qh_ps[:, nsl], wqb[:], ones[:], start=False, stop=True)
                        mm(kh_ps[:, nsl], wkb[:], ones[:], start=False, stop=True)
                        mm(vh_ps[:, nsl], wvb[:], ones_bf[:], start=False, stop=True)
                    nc.vector.tensor_copy(qhT_full[:, ssl], qh_ps[:])
                    nc.vector.tensor_copy(vhT_full[:, ssl], vh_ps[:])
                    for h in range(HPC):
                        psl = slice(h * DEPTH, (h + 1) * DEPTH)
                        nc.vector.tensor_copy(khT[h][:, ssl], kh_ps[psl, :])

                    # transpose projections into [t, d] layouts
                    nj = SH // 128
                    for jj in range(nj):
                        j = half * nj + jj
                        isl = slice(half * SH + jj * 128,
                                    half * SH + (jj + 1) * 128)
                        tq = strps.tile([128, 128], f32, tag="tq", name="tq")
                        nc.tensor.transpose(tq[:], qhT_full[:, isl], ident[:])
                        nc.vector.tensor_copy(qh_sd[:, j * P:(j + 1) * P], tq[:])
                        tv = strps.tile([128, 128], bf16, tag="tv", name="tv")
                        nc.tensor.transpose(tv[:], vhT_full[:, isl], ident_bf[:])
                        nc.scalar.copy(vh_sd[:, j * P:(j + 1) * P], tv[:])

        # ---------------- A path: metric tensor, ResLayers, plga ----------------
        with (
            tc.tile_pool(name="rl_w", bufs=1) as rlw,
            tc.tile_pool(name="a_sb", bufs=1) as asb,
            tc.tile_pool(name="a_ps", bufs=1, space="PSUM") as aps,
            tc.tile_pool(name="a_sc", bufs=2) as asc,
        ):
            w1_sb = [rlw.tile([DEPTH, A_DFF], f32, tag=f"w1_{i}", name=f"w1_{i}")
                     for i in range(NRES)]
            w2_sb = [rlw.tile([128, (A_DFF // 128) * DEPTH], f32, tag=f"w2_{i}",
                              name=f"w2_{i}") for i in range(NRES)]
            b1_sb = [rlw.tile([128, A_DFF // 128], f32, tag=f"b1_{i}",
                              name=f"b1_{i}") for i in range(NRES)]
            b2_sb = [rlw.tile([1, DEPTH], f32, tag=f"b2_{i}", name=f"b2_{i}")
                     for i in range(NRES)]
            g_sb = [rlw.tile([DEPTH, DEPTH], f32, tag=f"g_{i}", name=f"g_{i}")
                    for i in range(NRES)]
            be_sb = [rlw.tile([DEPTH, DEPTH], f32, tag=f"be_{i}", name=f"be_{i}")
                     for i in range(NRES)]
            for i in range(NRES):
                dma(w1_sb[i][:], w1_d[i])
                for c in range(A_DFF // 128):
                    dma(w2_sb[i][:, c * DEPTH:(c + 1) * DEPTH],
                        w2_d[i, c * 128:(c + 1) * 128, :])
                    dma(b1_sb[i][:, c:c + 1], b1_d[i, c])
                dma(b2_sb[i][:], b2_d[i])
                dma(g_sb[i][:], g_d[i])
                dma(be_sb[i][:], beta_d[i])
            avec_sb = rlw.tile([DEPTH, DEPTH], f32, tag="avec", name="avec")
            dma(avec_sb[:], avec_d[:, :])
            pw_sb = rlw.tile([DEPTH, 1], f32, tag="pw", name="pw")
            dma(pw_sb[:], pw_d[:, :])

            a_ps = aps.tile([128, 128], f32, tag="a_full", name="a_full")
            for j in range(NSC):
                jsl = slice(j * P, (j + 1) * P)
                mm(a_ps[:], qh_sd[:, jsl], qh_sd[:, jsl],
                   start=(j == 0), stop=(j == NSC - 1))

            for h in range(HPC):
                hsl = slice(h * DEPTH, (h + 1) * DEPTH)
                A_sb = asb.tile([DEPTH, DEPTH], f32, tag=f"A_{h}", name=f"A_{h}")
                nc.vector.tensor_copy(A_sb[:], a_ps[hsl, hsl])

                for i in range(NRES):
                    t_ps = aps.tile([DEPTH, DEPTH], f32, tag="t_ps", name="t_ps")
                    nc.tensor.transpose(t_ps[:], A_sb[:], ident[0:DEPTH, 0:DEPTH])
                    AinT = asc.tile([DEPTH, DEPTH], f32, tag="AinT", name="AinT")
                    nc.vector.tensor_copy(AinT[:], t_ps[:])

                    hdnT = asc.tile([128, (A_DFF // 128) * DEPTH], f32,
                                    tag="hdnT", name="hdnT")
                    for fc in range(A_DFF // 128):
                        hd_ps = aps.tile([128, DEPTH], f32, tag="hd_ps",
                                         name="hd_ps", bufs=2)
                        mm(hd_ps[:], w1_sb[i][:, fc * 128:(fc + 1) * 128], AinT[:],
                           start=True, stop=True)
                        nc.scalar.activation(hdnT[:, fc * DEPTH:(fc + 1) * DEPTH],
                                             hd_ps[:], Act.Relu,
                                             bias=b1_sb[i][:, fc:fc + 1])

                    h2_ps = aps.tile([DEPTH, DEPTH], f32, tag="h2_ps", name="h2_ps")
                    for fc in range(A_DFF // 128):
                        fsl = slice(fc * DEPTH, (fc + 1) * DEPTH)
                        mm(h2_ps[:], hdnT[:, fsl], w2_sb[i][:, fsl],
                           start=(fc == 0), stop=False)
                    mm(h2_ps[:], ones[:, 0:DEPTH], b2_sb[i][:],
                       start=False, stop=False)
                    mm(h2_ps[:], ident[0:DEPTH, 0:DEPTH], A_sb[:],
                       start=False, stop=True)

                    ssum = asc.tile([DEPTH, 1], f32, tag="ssum", name="ssum")
                    nc.vector.tensor_reduce(ssum[:], h2_ps[:], Ax.X, Alu.add)
                    mean = asc.tile([DEPTH, 1], f32, tag="mean", name="mean")
                    nc.vector.tensor_scalar_mul(mean[:], ssum[:], 1.0 / DEPTH)
                    xc = asc.tile([DEPTH, DEPTH], f32, tag="xc", name="xc")
                    nc.vector.tensor_scalar(xc[:], h2_ps[:], mean[:], None,
                                            Alu.subtract)
                    xsq = asc.tile([DEPTH, DEPTH], f32, tag="xsq", name="xsq")
                    ssq = asc.tile([DEPTH, 1], f32, tag="ssq", name="ssq")
                    nc.scalar.activation(xsq[:], xc[:], Act.Square,
                                         bias=zcol[0:DEPTH, :], accum_out=ssq[:])
                    lnv = asc.tile([DEPTH, 1], f32, tag="lnv", name="lnv")
                    nc.scalar.activation(lnv[:], ssq[:], Act.Ln,
                                         scale=1.0 / DEPTH, bias=lneps_c[:])
                    rstd = asc.tile([DEPTH, 1], f32, tag="rstd", name="rstd")
                    nc.scalar.activation(rstd[:], lnv[:], Act.Exp, scale=-0.5,
                                         bias=zcol[0:DEPTH, :])
                    A_sb = asb.tile([DEPTH, DEPTH], f32, tag=f"A_{h}_l{i}",
                                    name=f"A_{h}_l{i}")
                    nc.vector.scalar_tensor_tensor(A_sb[:], xc[:], rstd[:],
                                                   g_sb[i][:], Alu.mult, Alu.mult)
                    nc.vector.tensor_tensor(A_sb[:], A_sb[:], be_sb[i][:], Alu.add)

                dma(A_d[h], A_sb[:])

                absA = asc.tile([DEPTH, DEPTH], f32, tag="absA", name="absA")
                nc.vector.scalar_tensor_tensor(absA[:], A_sb[:], -1.0, A_sb[:],
                                               Alu.mult, Alu.max)
                L = asc.tile([DEPTH, DEPTH], f32, tag="L", name="L")
                nc.scalar.activation(L[:], absA[:], Act.Ln, bias=poweps_c[:])
                lt_ps = aps.tile([DEPTH, DEPTH], f32, tag="lt_ps", name="lt_ps")
                nc.tensor.transpose(lt_ps[:], L[:], ident[0:DEPTH, 0:DEPTH])
                PT = asc.tile([DEPTH, DEPTH], f32, tag="PT", name="PT")
                nc.vector.tensor_scalar(PT[:], lt_ps[:], pw_sb[:], None, Alu.mult)
                ApT = asc.tile([DEPTH, DEPTH], f32, tag="ApT", name="ApT")
                nc.scalar.activation(ApT[:], PT[:], Act.Exp, bias=zcol[0:DEPTH, :])
                wApT = asc.tile([DEPTH, DEPTH], f32, tag="wApT", name="wApT")
                nc.vector.tensor_tensor(wApT[:], ApT[:], avec_sb[:], Alu.mult)
                av_sb = asc.tile([DEPTH, 1], f32, tag=f"av_{h}", name=f"av_{h}")
                nc.vector.tensor_reduce(av_sb[:], wApT[:], Ax.X, Alu.add)
                dma(av_d[h], av_sb[:])

                # scaled q head (bf16) for the E / E^T matmuls
                nc.vector.tensor_scalar(qhsc[h][:], qhT_full[hsl, :], av_sb[:],
                                        None, Alu.mult)

        # ---------------- main attention loops ----------------
        with (
            tc.tile_pool(name="e_ps", bufs=2, space="PSUM") as eps_pool,
            tc.tile_pool(name="et_ps", bufs=2, space="PSUM") as etps_pool,
            tc.tile_pool(name="hn_ps", bufs=2, space="PSUM") as hnps_pool,
            tc.tile_pool(name="it_ps", bufs=1, space="PSUM") as itps_pool,
            tc.tile_pool(name="bc_ps", bufs=1, space="PSUM") as bcps_pool,
            tc.tile_pool(name="row_sb", bufs=3) as row_pool,
            tc.tile_pool(name="sm_sb", bufs=3) as sm_pool,
            tc.tile_pool(name="tp_sb", bufs=4) as tp_pool,
        ):
            for h in range(HPC):
                for sb in range(NTB):
                    # ---- s-orientation pass: 4 row-chunks of 128 s each
                    for c4 in range(4):
                        sc = sb * 4 + c4
                        ssl = slice(sc * 128, (sc + 1) * 128)
                        icol = inv_all[:, h * NSC + sc: h * NSC + sc + 1]
                        e_sb = row_pool.tile([128, s_len], f32, tag="e_sb",
                                             name="e_sb")
                        for tb in range(NTB):
                            nsl = slice(tb * 512, (tb + 1) * 512)
                            e_ps = eps_pool.tile([128, 512], f32, tag="e_ps",
                                                 name="e_ps")
                            mm(e_ps[:], qhsc[h][:, ssl], khT[h][:, nsl],
                               start=True, stop=True)
                            if tb % 2 == 0:
                                nc.vector.tensor_copy(e_sb[:, nsl], e_ps[:])
                            else:
                                nc.scalar.copy(e_sb[:, nsl], e_ps[:])
                        dma(E_d[h, ssl, :], e_sb[:])

                        ep_sb = row_pool.tile([128, s_len], f32, tag="ep_sb",
                                              name="ep_sb")
                        nc.vector.scalar_tensor_tensor(ep_sb[:], e_sb[:], SLOPE,
                                                       e_sb[:], Alu.mult, Alu.max)
                        if has_bias_row:
                            nc.vector.tensor_tensor(ep_sb[:], ep_sb[:],
                                                    bias_bc[:], Alu.add)
                        dma(Ep_d[h, ssl, :], ep_sb[:])

                        p_sb = sm_pool.tile([128, s_len], f32, tag="p_sb",
                                            name="p_sb")
                        d_col = sm_pool.tile([128, 1], f32, tag="d_col",
                                             name="d_col")
                        nc.scalar.activation(p_sb[:], ep_sb[:], Act.Exp,
                                             bias=zcol[:], accum_out=d_col[:])
                        nc.vector.reciprocal(icol, d_col[:])
                        attv_sb = sm_pool.tile([128, s_len], f32, tag="attv_sb",
                                               name="attv_sb")
                        nc.vector.tensor_scalar(attv_sb[:], p_sb[:], icol,
                                                None, Alu.mult)
                        dma(attv_d[h, ssl, :], attv_sb[:])

                    # ---- transposed pass for this 512-wide s-block
                    bsl = slice(sb * 512, (sb + 1) * 512)
                    inv_row = tp_pool.tile([1, 512], f32, tag="inv_row",
                                           name="inv_row")
                    for c4 in range(4):
                        sc = sb * 4 + c4
                        it_ps = itps_pool.tile([1, 128], f32, tag="it_ps",
                                               name="it_ps")
                        nc.tensor.transpose(
                            it_ps[:], inv_all[:, h * NSC + sc: h * NSC + sc + 1],
                            ident[:])
                        nc.vector.tensor_copy(inv_row[:, c4 * 128:(c4 + 1) * 128],
                                              it_ps[:])
                    bc_ps = bcps_pool.tile([DEPTH, 512], f32, tag="bc_ps",
                                           name="bc_ps")
                    mm(bc_ps[:], ones[:, 0:DEPTH], inv_row[:], start=True,
                       stop=True)
                    inv_bc = tp_pool.tile([DEPTH, 512], f32, tag="inv_bc",
                                          name="inv_bc")
                    nc.vector.tensor_copy(inv_bc[:], bc_ps[:])

                    hn_ps = hnps_pool.tile([DEPTH, 512], f32, tag="hn_ps",
                                           name="hn_ps")
                    for tb in range(NSC):
                        tsl = slice(tb * 128, (tb + 1) * 128)
                        et_ps = etps_pool.tile([128, 512], f32, tag="et_ps",
                                               name="et_ps")
                        mm(et_ps[:], khT[h][:, tsl], qhsc[h][:, bsl],
                           start=True, stop=True)
                        et_sb = tp_pool.tile([128, 512], f32, tag="et_sb",
                                             name="et_sb")
                        nc.scalar.copy(et_sb[:], et_ps[:])
                        lre = tp_pool.tile([128, 512], f32, tag="lre", name="lre")
                        nc.vector.scalar_tensor_tensor(lre[:], et_sb[:], SLOPE,
                                                       et_sb[:], Alu.mult, Alu.max)
                        if has_bias_row:
                            nc.vector.tensor_scalar(
                                lre[:], lre[:], biascol_sb[:, tb:tb + 1], None,
                                Alu.add)
                        pT = tp_pool.tile([128, 512], bf16, tag="pT", name="pT")
                        nc.scalar.activation(pT[:], lre[:], Act.Exp, bias=zcol[:])
                        mm(hn_ps[:], vh_sd[:, tb * P + h * DEPTH:
                                           tb * P + (h + 1) * DEPTH], pT[:],
                           start=(tb == 0), stop=(tb == NSC - 1))
                    nc.vector.scalar_tensor_tensor(
                        HnT[h * DEPTH:(h + 1) * DEPTH, bsl], hn_ps[:], 1.0,
                        inv_bc[:], Alu.mult, Alu.mult)

            # ---------------- output projection (partial) ----------------
            for sc in range(NSC):
                ssl = slice(sc * 128, (sc + 1) * 128)
                o_sb = row_pool.tile([128, D], f32, tag="o_sb", name="o_sb")
                for nb in range(D // 512):
                    nsl = slice(nb * 512, (nb + 1) * 512)
                    o_ps = eps_pool.tile([128, 512], f32, tag="e_ps", name="o_ps")
                    mm(o_ps[:], HnT[:, ssl], wo_sb[:, nsl], start=True, stop=True)
                    nc.vector.tensor_copy(o_sb[:, nsl], o_ps[:])
                dma(op_d[ssl, :], o_sb[:])

    nc.compile()
    return nc


def _prepare_inputs(q, k, v, mask, wq_w, wq_b, wk_w, wk_b, wv_w, wv_b, wo_w, wo_b,
                    rl_w1, rl_b1, rl_w2, rl_b2, rl_g, rl_beta, pw, a_vec, ba,
                    s_len):
    """Build the 8 per-core input maps (host-side sharding / layout prep)."""
    import ml_dtypes
    bf = ml_dtypes.bfloat16
    f = lambda x: np.ascontiguousarray(np.asarray(x, dtype=F32))
    fb = lambda x: np.ascontiguousarray(np.asarray(x, dtype=F32).astype(bf))
    qT = f(np.asarray(q, dtype=F32).reshape(s_len, D).T)
    kT = f(np.asarray(k, dtype=F32).reshape(s_len, D).T)
    vT = fb(np.asarray(v, dtype=F32).reshape(s_len, D).T)
    ident = np.eye(128, dtype=F32)
    ident_bf = np.eye(128).astype(bf)
    b1c = f(np.asarray(rl_b1).reshape(NRES, A_DFF // 128, 128, 1))
    b2r = f(np.asarray(rl_b2).reshape(NRES, 1, DEPTH))
    g_bc = f(np.broadcast_to(np.asarray(rl_g)[:, None, :], (NRES, DEPTH, DEPTH)))
    beta_bc = f(np.broadcast_to(np.asarray(rl_beta)[:, None, :],
                                (NRES, DEPTH, DEPTH)))
    avec_bc = f(np.broadcast_to(np.asarray(a_vec)[None, :], (DEPTH, DEPTH)))
    pw_col = f(np.asarray(pw).reshape(DEPTH, 1))
    bias_row = (np.asarray(ba, dtype=F32)
                + np.asarray(mask, dtype=F32).reshape(1, s_len) * F32(-1e9))
    has_bias_row = bool(np.any(bias_row != 0))

    in_maps = []
    for c in range(N_CORES):
        cols = slice(c * P, (c + 1) * P)
        m = {
            "qT": qT, "kT": kT, "vT": vT,
            "wq_s": f(np.asarray(wq_w)[:, cols]),
            "wk_s": f(np.asarray(wk_w)[:, cols]),
            "wv_s": fb(np.asarray(wv_w, dtype=F32)[:, cols]),
            "wq_bs": f(np.asarray(wq_b)[cols].reshape(1, P)),
            "wk_bs": f(np.asarray(wk_b)[cols].reshape(1, P)),
            "wv_bs": fb(np.asarray(wv_b, dtype=F32)[cols].reshape(1, P)),
            "wo_s": fb(np.asarray(wo_w, dtype=F32)[cols, :]),
            "rl_w1": f(rl_w1), "rl_b1c": b1c, "rl_w2": f(rl_w2), "rl_b2r": b2r,
            "rl_g_bc": g_bc, "rl_beta_bc": beta_bc,
            "avec_bc": avec_bc, "pw_col": pw_col,
            "ident": ident, "ident_bf": ident_bf,
        }
        if has_bias_row:
            m["bias_row"] = f(bias_row)
            m["bias_colT"] = f(bias_row.reshape(-1, 128).T)
        in_maps.append(m)
    return in_maps, has_bias_row


def _ensure_trn_backend():
    """The device run needs jax's axon TRN backend. If the surrounding process
    pinned JAX_PLATFORMS=cpu (e.g. to run the jax reference), lift that before
    jax/concourse initialize, and reset jax backends if they were already
    initialized without the TRN devices."""
    import os
    import sys
    jp = os.environ.get("JAX_PLATFORMS")
    if jp is not None and "axon" not in jp and jp != "":
        os.environ.pop("JAX_PLATFORMS", None)
    if "jax" in sys.modules:
        try:
            import jax
            if not any("NC_" in str(d) for d in jax.devices()):
                jax.clear_backends()
        except Exception:
            try:
                import jax
                jax.clear_backends()
            except Exception:
                pass


def kernel(q, k, v, mask, wq_w, wq_b, wk_w, wk_b, wv_w, wv_b, wo_w, wo_b,
           rl_w1, rl_b1, rl_w2, rl_b2, rl_g, rl_beta, pw, a_vec, ba,
           _trace=False):
    _ensure_trn_backend()
    from concourse.bass_utils import run_bass_kernel_spmd

    s_len = int(np.asarray(q).shape[1])
    in_maps, has_bias_row = _prepare_inputs(
        q, k, v, mask, wq_w, wq_b, wk_w, wk_b, wv_w, wv_b, wo_w, wo_b,
        rl_w1, rl_b1, rl_w2, rl_b2, rl_g, rl_beta, pw, a_vec, ba, s_len)

    key = (s_len, has_bias_row)
    if key not in _CACHE:
        _CACHE[key] = build_program(s_len, has_bias_row)
    nc = _CACHE[key]

    res = run_bass_kernel_spmd(nc, in_maps, core_ids=list(range(N_CORES)),
                               trace=_trace)

    E = np.empty((1, H, s_len, s_len), dtype=F32)
    Ep = np.empty((1, H, s_len, s_len), dtype=F32)
    attv = np.empty((1, H, s_len, s_len), dtype=F32)
    A = np.empty((1, H, DEPTH, DEPTH), dtype=F32)
    avAp = np.empty((1, H, DEPTH), dtype=F32)
    out_acc = np.zeros((s_len, D), dtype=F32)
    for c in range(N_CORES):
        r = res.results[c]
        E[0, c * HPC:(c + 1) * HPC] = r["E_out"]
        Ep[0, c * HPC:(c + 1) * HPC] = r["Ep_out"]
        attv[0, c * HPC:(c + 1) * HPC] = r["attv_out"]
        A[0, c * HPC:(c + 1) * HPC] = r["A_out"]
        avAp[0, c * HPC:(c + 1) * HPC] = r["av_out"].reshape(HPC, DEPTH)
        out_acc += r["out_part"]
    out = (out_acc + np.asarray(wo_b, dtype=F32)).reshape(1, s_len, D)

    pw_out = np.asarray(pw, dtype=F32)
    ba_out = np.asarray(ba, dtype=F32)
    ret = (out, E, A, pw_out, attv, ba_out, avAp, Ep)
    if _trace:
        return ret, res
    return ret
